# revision 11
# baseline (speedup 1.0000x reference)
"""Tensor-parallel multi-head attention for Trainium2 (8 NeuronCores).

Problem: nn_MultiHeadAttention (B=2, N=2048, C=1024, H=16, D=64), fp32.

Sharding: core = batch * 4 + head_group; each core handles 1 batch and 4
heads (tensor-parallel over heads, data-parallel over batch). Each core
computes its heads' QKV projections, attention, and a *partial* output
projection (its 256 rows of w_proj); the host sums the 4 partials per
batch and adds b_proj.

Kernel math notes:
  - x is transposed on the host to xT [C, N] (feature-major) so all
    projections contract over partitions.
  - Scores are computed transposed: sT[m, n] = k[m]·q[n] with keys m on
    partitions -- so P@V needs no on-chip transposes. Two heads run
    concurrently on the PE array via row-tiling (K=64 each).
  - Softmax: no max-subtraction (logits ~ N(0,1), exp is fp32-safe);
    denominator obtained by appending a ones-column to V (row 64 of the
    attention-output accumulator); probabilities are normalized after
    the P@V matmul via a reciprocal + DMA partition-broadcast multiply.
  - k-bias is mathematically softmax-invariant and dropped; v-bias is
    added to the attention output (softmax rows sum to 1); q-bias is
    applied at QKV eviction; proj-bias is added on the host.
"""

import numpy as np
from contextlib import ExitStack

P = 128
C = 1024
D = 64
N_CORES = 8

_BUILT = {}
TRACE = False   # set True (e.g. from test.py) to capture an NTFF profile
LAST_RESULTS = None  # BassKernelResults of the most recent kernel() call


def _build(n_tok, debug=False):
    import concourse.bass as bass
    import concourse.mybir as mybir
    import concourse.tile as tile
    from concourse import bacc
    from concourse.bass import ts

    fp32 = mybir.dt.float32
    Exp = mybir.ActivationFunctionType.Exp
    mult = mybir.AluOpType.mult

    NC5 = n_tok // 512  # 512-wide query chunks
    MC = n_tok // 128   # 128-wide key chunks
    CC = C // P         # contraction chunks for projections

    nc = bacc.Bacc("TRN2", target_bir_lowering=False, debug=debug)

    xt_d = nc.dram_tensor("xt", [C, n_tok], fp32, kind="ExternalInput").ap()
    wqk_d = nc.dram_tensor("w_qk", [C, 512], fp32, kind="ExternalInput").ap()
    wv_d = nc.dram_tensor("w_v", [C, 256], fp32, kind="ExternalInput").ap()
    wp_d = nc.dram_tensor("w_p", [256, C], fp32, kind="ExternalInput").ap()
    bq_d = nc.dram_tensor("b_q", [256], fp32, kind="ExternalInput").ap()
    bv_d = nc.dram_tensor("b_v2", [64, 4], fp32, kind="ExternalInput").ap()
    out_d = nc.dram_tensor("out", [n_tok, C], fp32, kind="ExternalOutput").ap()

    with tile.TileContext(nc) as tc, ExitStack() as ctx:
        persist = ctx.enter_context(tc.tile_pool(name="persist", bufs=1))
        p_pool = ctx.enter_context(tc.tile_pool(name="p_pool", bufs=3))
        ev_pool = ctx.enter_context(tc.tile_pool(name="ev_pool", bufs=2))
        ph1_cm = tc.tile_pool(name="ph1", bufs=1)
        ph1 = ph1_cm.__enter__()
        s_pool = ctx.enter_context(tc.tile_pool(name="s", bufs=2, space="PSUM"))
        sm_pool = ctx.enter_context(tc.tile_pool(name="sm", bufs=2, space="PSUM"))
        o2_pool = ctx.enter_context(tc.tile_pool(name="o2", bufs=2, space="PSUM"))
        dram_pool = ctx.enter_context(tc.tile_pool(name="dram", bufs=4, space="DRAM"))

        xt = ph1.tile([P, CC, n_tok], fp32)
        wqk = ph1.tile([P, CC, 512], fp32)
        wv = ph1.tile([P, CC, 256], fp32)
        bq = ph1.tile([P, 2], fp32)
        wp = persist.tile([P, 2, C], fp32)
        bv = persist.tile([64, 4], fp32)
        qk = persist.tile([P, 4, n_tok], fp32)   # jc: 0,1 = qT pairs, 2,3 = kT pairs
        vsb = persist.tile([P, MC, 4, 65], fp32)  # token-major V + ones column
        o2n = persist.tile([P, 2, n_tok], fp32)   # normalized attn out, feature-major

        nc.sync.dma_start(xt[:], xt_d.rearrange("(co p) n -> p co n", p=P))
        nc.sync.dma_start(wqk[:], wqk_d.rearrange("(co p) j -> p co j", p=P))
        nc.sync.dma_start(wv[:], wv_d.rearrange("(co p) j -> p co j", p=P))
        nc.sync.dma_start(wp[:], wp_d.rearrange("(pc p) e -> p pc e", p=P))
        nc.sync.dma_start(bq[:], bq_d.rearrange("(c p) -> p c", p=P))
        nc.sync.dma_start(bv[:], bv_d)
        nc.vector.memset(vsb[:, :, :, 64:65], 1.0)

        def emit_qk(pc):
            # kT then qT for this head pair (k first: scores need all keys)
            for jc, wcol in ((2 + pc, 256 + pc * 128), (pc, pc * 128)):
                for n5 in range(NC5):
                    ps = sm_pool.tile([P, 512], fp32, tag="sm")
                    for cc in range(CC):
                        nc.tensor.matmul(
                            ps[:],
                            wqk[:, cc, wcol:wcol + 128],
                            xt[:, cc, ts(n5, 512)],
                            start=(cc == 0),
                            stop=(cc == CC - 1),
                        )
                    if jc < 2:  # q: add bias
                        nc.vector.tensor_scalar_add(
                            qk[:, jc, ts(n5, 512)], ps[:], bq[:, jc:jc + 1]
                        )
                    else:  # k: bias dropped (softmax-invariant)
                        nc.vector.tensor_copy(
                            out=qk[:, jc, ts(n5, 512)], in_=ps[:]
                        )

        def emit_v():
            for nt in range(MC):
                psv = sm_pool.tile([P, 256], fp32, tag="sm")
                for cc in range(CC):
                    nc.tensor.matmul(
                        psv[:],
                        xt[:, cc, ts(nt, 128)],
                        wv[:, cc, :],
                        start=(cc == 0),
                        stop=(cc == CC - 1),
                    )
                nc.vector.tensor_copy(
                    out=vsb[:, nt, :, 0:64],
                    in_=psv[:].rearrange("p (h d) -> p h d", d=64),
                )

        def emit_attn(pc):
            for n5 in range(NC5):
                o2a = o2_pool.tile([65, 512], fp32, tag="o2")
                o2b = o2_pool.tile([65, 512], fp32, tag="o2")
                for mc in range(MC):
                    s = s_pool.tile([P, 1024], fp32, tag="s")
                    nc.tensor.matmul(
                        s[:, 0:512],
                        qk[0:64, 2 + pc, ts(mc, 128)],
                        qk[0:64, pc, ts(n5, 512)],
                        start=True, stop=True, tile_position=(0, 0),
                    )
                    nc.tensor.matmul(
                        s[:, 512:1024],
                        qk[64:128, 2 + pc, ts(mc, 128)],
                        qk[64:128, pc, ts(n5, 512)],
                        start=True, stop=True, tile_position=(64, 0),
                    )
                    pab = p_pool.tile([P, 1024], fp32, tag="pab")
                    nc.scalar.activation(pab[:], s[:], Exp, scale=0.125)
                    nc.tensor.matmul(
                        o2a[:], vsb[:, mc, 2 * pc, 0:65], pab[:, 0:512],
                        start=(mc == 0), stop=(mc == MC - 1),
                    )
                    nc.tensor.matmul(
                        o2b[:], vsb[:, mc, 2 * pc + 1, 0:65], pab[:, 512:1024],
                        start=(mc == 0), stop=(mc == MC - 1),
                    )
                for hl, o2 in ((0, o2a), (1, o2b)):
                    rec = ev_pool.tile([1, 512], fp32, tag="rec")
                    nc.vector.reciprocal(rec[:], o2[64:65, :])
                    # partition-broadcast the reciprocal row via a DRAM bounce
                    rd = dram_pool.tile([1, 512], fp32, tag="rd")
                    nc.sync.dma_start(rd[:], rec[:])
                    rb = ev_pool.tile([64, 512], fp32, tag="rb")
                    rd_bcast = bass.AP(
                        tensor=rd.tensor, offset=rd.offset, ap=[[0, 64], [1, 512]]
                    )
                    nc.sync.dma_start(rb[:], rd_bcast)
                    if hl == 0:
                        dst = o2n[0:64, pc, ts(n5, 512)]
                        nc.vector.scalar_tensor_tensor(
                            dst, o2[0:64, :], 1.0, rb[:], op0=mult, op1=mult
                        )
                        nc.vector.tensor_scalar_add(
                            dst, dst, bv[:, 2 * pc:2 * pc + 1]
                        )
                    else:
                        stg = ev_pool.tile([64, 512], fp32, tag="stg")
                        nc.vector.scalar_tensor_tensor(
                            stg[:], o2[0:64, :], 1.0, rb[:], op0=mult, op1=mult
                        )
                        nc.vector.tensor_scalar_add(
                            stg[:], stg[:], bv[:, 2 * pc + 1:2 * pc + 2]
                        )
                        nc.sync.dma_start(o2n[64:128, pc, ts(n5, 512)], stg[:])

        def emit_proj(ostg):
            for nt in range(MC):
                po = ostg.tile([P, 1024], fp32, tag="po")
                for ec in range(2):
                    pp = sm_pool.tile([P, 512], fp32, tag="sm")
                    for pc in range(2):
                        nc.tensor.matmul(
                            pp[:],
                            o2n[:, pc, ts(nt, 128)],
                            wp[:, pc, ts(ec, 512)],
                            start=(pc == 0),
                            stop=(pc == 1),
                        )
                    nc.vector.tensor_copy(out=po[:, ts(ec, 512)], in_=pp[:])
                nc.sync.dma_start(out_d[ts(nt, 128), :], po[:])

        emit_qk(0)
        emit_v()
        emit_attn(0)
        emit_qk(1)
        ph1_cm.__exit__(None, None, None)  # free xt/weights space for ostg
        emit_attn(1)
        with tc.tile_pool(name="ostg", bufs=2) as ostg:
            emit_proj(ostg)

    nc.compile()
    return nc


def _get_built(n_tok):
    if n_tok not in _BUILT:
        _BUILT[n_tok] = _build(n_tok)
    return _BUILT[n_tok]


def make_in_map(x_b, w_qkv, b_qkv, w_proj, g):
    """Per-core input shards: batch slice x_b, head-group g (4 heads)."""
    f = np.float32
    cq = slice(g * 256, g * 256 + 256)
    ck = slice(C + g * 256, C + g * 256 + 256)
    cv = slice(2 * C + g * 256, 2 * C + g * 256 + 256)
    return {
        "xt": np.ascontiguousarray(np.asarray(x_b, f).T),
        "w_qk": np.ascontiguousarray(
            np.concatenate(
                [np.asarray(w_qkv[:, cq], f), np.asarray(w_qkv[:, ck], f)], axis=1
            )
        ),
        "w_v": np.ascontiguousarray(np.asarray(w_qkv[:, cv], f)),
        "w_p": np.ascontiguousarray(np.asarray(w_proj[g * 256:(g + 1) * 256, :], f)),
        "b_q": np.ascontiguousarray(np.asarray(b_qkv[cq], f)),
        "b_v2": np.ascontiguousarray(np.asarray(b_qkv[cv], f).reshape(4, 64).T),
    }


def kernel(x, w_qkv, b_qkv, w_proj, b_proj):
    from concourse.bass_utils import run_bass_kernel_spmd

    x = np.asarray(x, np.float32)
    B, n_tok, _ = x.shape
    nc = _get_built(n_tok)

    in_maps = [
        make_in_map(x[core // 4], w_qkv, b_qkv, w_proj, core % 4)
        for core in range(N_CORES)
    ]
    res = run_bass_kernel_spmd(
        nc, in_maps, core_ids=list(range(N_CORES)), trace=TRACE
    )
    global LAST_RESULTS
    LAST_RESULTS = res
    outs = [r["out"] for r in res.results]
    bp = np.asarray(b_proj, np.float32)
    full = np.stack(
        [
            outs[4 * b] + outs[4 * b + 1] + outs[4 * b + 2] + outs[4 * b + 3] + bp
            for b in range(B)
        ]
    )
    return full.astype(np.float32)


# revision 13
# speedup vs baseline: 2.3789x; 2.3789x over previous
"""Tensor-parallel multi-head attention for Trainium2 (8 NeuronCores).

Problem: nn_MultiHeadAttention (B=2, N=2048, C=1024, H=16, D=64), fp32.

Sharding: core = batch * 4 + head_group; each core handles 1 batch and 4
heads (tensor-parallel over heads, data-parallel over batch). Each core
computes its heads' QKV projections, attention, and a *partial* output
projection (its 256 rows of w_proj); the host sums the 4 partials per
batch and adds b_proj.

Kernel math notes:
  - x is transposed on the host to xT [C, N] (feature-major) so all
    projections contract over partitions.
  - Scores are computed transposed: sT[m, n] = k[m]·q[n] with keys m on
    partitions -- so P@V needs no on-chip transposes. Two heads run
    concurrently on the PE array via row-tiling (K=64 each).
  - Softmax: no max-subtraction (logits ~ N(0,1), exp is fp32-safe);
    denominator obtained by appending a ones-column to V (row 64 of the
    attention-output accumulator); probabilities are normalized after
    the P@V matmul via a reciprocal + DMA partition-broadcast multiply.
  - k-bias is mathematically softmax-invariant and dropped; v-bias is
    added to the attention output (softmax rows sum to 1); q-bias is
    applied at QKV eviction; proj-bias is added on the host.
"""

import numpy as np
from contextlib import ExitStack

P = 128
C = 1024
D = 64
N_CORES = 8

_BUILT = {}
TRACE = False   # set True (e.g. from test.py) to capture an NTFF profile
LAST_RESULTS = None  # BassKernelResults of the most recent kernel() call


def _build(n_tok, debug=False):
    import concourse.bass as bass
    import concourse.mybir as mybir
    import concourse.tile as tile
    from concourse import bacc
    from concourse.bass import ts

    fp32 = mybir.dt.float32
    fp32r = mybir.dt.float32r
    Exp = mybir.ActivationFunctionType.Exp
    mult = mybir.AluOpType.mult

    NC5 = n_tok // 512  # 512-wide query chunks
    MC = n_tok // 128   # 128-wide key chunks
    CC = C // P         # contraction chunks for projections

    nc = bacc.Bacc("TRN2", target_bir_lowering=False, debug=debug)

    xt_d = nc.dram_tensor("xt", [C, n_tok], fp32, kind="ExternalInput").ap()
    wqk_d = nc.dram_tensor("w_qk", [C, 512], fp32, kind="ExternalInput").ap()
    wv_d = nc.dram_tensor("w_v", [C, 256], fp32, kind="ExternalInput").ap()
    wp_d = nc.dram_tensor("w_p", [256, C], fp32, kind="ExternalInput").ap()
    bq_d = nc.dram_tensor("b_q", [256], fp32, kind="ExternalInput").ap()
    bv_d = nc.dram_tensor("b_v2", [64, 4], fp32, kind="ExternalInput").ap()
    out_d = nc.dram_tensor("out", [n_tok, C], fp32, kind="ExternalOutput").ap()

    with tile.TileContext(nc) as tc, ExitStack() as ctx:
        persist = ctx.enter_context(tc.tile_pool(name="persist", bufs=1))
        p_pool = ctx.enter_context(tc.tile_pool(name="p_pool", bufs=3))
        ev_pool = ctx.enter_context(tc.tile_pool(name="ev_pool", bufs=2))
        ph1_cm = tc.tile_pool(name="ph1", bufs=1)
        ph1 = ph1_cm.__enter__()
        s_pool = ctx.enter_context(tc.tile_pool(name="s", bufs=2, space="PSUM"))
        sm_pool = ctx.enter_context(tc.tile_pool(name="sm", bufs=2, space="PSUM"))
        o2_pool = ctx.enter_context(tc.tile_pool(name="o2", bufs=2, space="PSUM"))
        dram_pool = ctx.enter_context(tc.tile_pool(name="dram", bufs=4, space="DRAM"))

        xt = ph1.tile([P, CC, n_tok], fp32r)
        wqk = ph1.tile([P, CC, 512], fp32r)
        wv = ph1.tile([P, CC, 256], fp32r)
        bq = ph1.tile([P, 2], fp32)
        wp = persist.tile([P, 2, C], fp32r)
        bv = persist.tile([64, 4], fp32)
        qk = persist.tile([P, 4, n_tok], fp32r)   # jc: 0,1 = qT pairs, 2,3 = kT pairs
        vsb = persist.tile([P, MC, 4, 65], fp32r)  # token-major V + ones column
        o2n = persist.tile([P, 2, n_tok], fp32r)   # normalized attn out, feature-major

        nc.gpsimd.dma_start(xt[:], xt_d.rearrange("(co p) n -> p co n", p=P))
        nc.gpsimd.dma_start(wqk[:], wqk_d.rearrange("(co p) j -> p co j", p=P))
        nc.gpsimd.dma_start(wv[:], wv_d.rearrange("(co p) j -> p co j", p=P))
        nc.gpsimd.dma_start(wp[:], wp_d.rearrange("(pc p) e -> p pc e", p=P))
        nc.sync.dma_start(bq[:], bq_d.rearrange("(c p) -> p c", p=P))
        nc.sync.dma_start(bv[:], bv_d)
        ones = persist.tile([P, 1], fp32)
        nc.vector.memset(ones[:], 1.0)
        nc.vector.tensor_copy(
            out=vsb[:, :, :, 64:65],
            in_=ones[:, None, :, None].to_broadcast((P, MC, 4, 1)),
        )

        def emit_qk(pc):
            # kT then qT for this head pair (k first: scores need all keys)
            for jc, wcol in ((2 + pc, 256 + pc * 128), (pc, pc * 128)):
                for n5 in range(NC5):
                    ps = sm_pool.tile([P, 512], fp32, tag="sm")
                    for cc in range(CC):
                        nc.tensor.matmul(
                            ps[:],
                            wqk[:, cc, wcol:wcol + 128],
                            xt[:, cc, ts(n5, 512)],
                            start=(cc == 0),
                            stop=(cc == CC - 1),
                        )
                    if jc < 2:  # q: add bias
                        nc.vector.tensor_scalar_add(
                            qk[:, jc, ts(n5, 512)], ps[:], bq[:, jc:jc + 1]
                        )
                    else:  # k: bias dropped (softmax-invariant)
                        nc.vector.tensor_copy(
                            out=qk[:, jc, ts(n5, 512)], in_=ps[:]
                        )

        def emit_v():
            for nt in range(MC):
                psv = sm_pool.tile([P, 256], fp32, tag="sm")
                for cc in range(CC):
                    nc.tensor.matmul(
                        psv[:],
                        xt[:, cc, ts(nt, 128)],
                        wv[:, cc, :],
                        start=(cc == 0),
                        stop=(cc == CC - 1),
                    )
                nc.vector.tensor_copy(
                    out=vsb[:, nt, :, 0:64],
                    in_=psv[:].rearrange("p (h d) -> p h d", d=64),
                )

        def emit_attn(pc):
            for n5 in range(NC5):
                o2a = o2_pool.tile([65, 512], fp32, tag="o2")
                o2b = o2_pool.tile([65, 512], fp32, tag="o2")
                for mc in range(MC):
                    s = s_pool.tile([P, 1024], fp32, tag="s")
                    nc.tensor.matmul(
                        s[:, 0:512],
                        qk[0:64, 2 + pc, ts(mc, 128)],
                        qk[0:64, pc, ts(n5, 512)],
                        start=True, stop=True, tile_position=(0, 0),
                    )
                    nc.tensor.matmul(
                        s[:, 512:1024],
                        qk[64:128, 2 + pc, ts(mc, 128)],
                        qk[64:128, pc, ts(n5, 512)],
                        start=True, stop=True, tile_position=(64, 0),
                    )
                    pab = p_pool.tile([P, 1024], fp32r, tag="pab")
                    nc.scalar.activation(pab[:], s[:], Exp, scale=0.125)
                    nc.tensor.matmul(
                        o2a[:], vsb[:, mc, 2 * pc, 0:65], pab[:, 0:512],
                        start=(mc == 0), stop=(mc == MC - 1),
                    )
                    nc.tensor.matmul(
                        o2b[:], vsb[:, mc, 2 * pc + 1, 0:65], pab[:, 512:1024],
                        start=(mc == 0), stop=(mc == MC - 1),
                    )
                for hl, o2 in ((0, o2a), (1, o2b)):
                    rec = ev_pool.tile([1, 512], fp32, tag="rec")
                    nc.vector.reciprocal(rec[:], o2[64:65, :])
                    # partition-broadcast the reciprocal row via a DRAM bounce
                    rd = dram_pool.tile([1, 512], fp32, tag="rd")
                    nc.sync.dma_start(rd[:], rec[:])
                    rb = ev_pool.tile([64, 512], fp32, tag="rb")
                    rd_bcast = bass.AP(
                        tensor=rd.tensor, offset=rd.offset, ap=[[0, 64], [1, 512]]
                    )
                    nc.sync.dma_start(rb[:], rd_bcast)
                    if hl == 0:
                        dst = o2n[0:64, pc, ts(n5, 512)]
                        nc.vector.scalar_tensor_tensor(
                            dst, o2[0:64, :], 1.0, rb[:], op0=mult, op1=mult
                        )
                        nc.vector.tensor_scalar_add(
                            dst, dst, bv[:, 2 * pc:2 * pc + 1]
                        )
                    else:
                        stg = ev_pool.tile([64, 512], fp32r, tag="stg")
                        nc.vector.scalar_tensor_tensor(
                            stg[:], o2[0:64, :], 1.0, rb[:], op0=mult, op1=mult
                        )
                        nc.vector.tensor_scalar_add(
                            stg[:], stg[:], bv[:, 2 * pc + 1:2 * pc + 2]
                        )
                        nc.sync.dma_start(o2n[64:128, pc, ts(n5, 512)], stg[:])

        def emit_proj(ostg):
            for nt in range(MC):
                po = ostg.tile([P, 1024], fp32, tag="po")
                for ec in range(2):
                    pp = sm_pool.tile([P, 512], fp32, tag="sm")
                    for pc in range(2):
                        nc.tensor.matmul(
                            pp[:],
                            o2n[:, pc, ts(nt, 128)],
                            wp[:, pc, ts(ec, 512)],
                            start=(pc == 0),
                            stop=(pc == 1),
                        )
                    nc.vector.tensor_copy(out=po[:, ts(ec, 512)], in_=pp[:])
                nc.sync.dma_start(out_d[ts(nt, 128), :], po[:])

        emit_qk(0)
        emit_v()
        emit_attn(0)
        emit_qk(1)
        ph1_cm.__exit__(None, None, None)  # free xt/weights space for ostg
        emit_attn(1)
        with tc.tile_pool(name="ostg", bufs=2) as ostg:
            emit_proj(ostg)

    nc.compile()
    return nc


def _get_built(n_tok):
    if n_tok not in _BUILT:
        _BUILT[n_tok] = _build(n_tok)
    return _BUILT[n_tok]


def make_in_map(x_b, w_qkv, b_qkv, w_proj, g):
    """Per-core input shards: batch slice x_b, head-group g (4 heads)."""
    f = np.float32
    cq = slice(g * 256, g * 256 + 256)
    ck = slice(C + g * 256, C + g * 256 + 256)
    cv = slice(2 * C + g * 256, 2 * C + g * 256 + 256)
    return {
        "xt": np.ascontiguousarray(np.asarray(x_b, f).T),
        "w_qk": np.ascontiguousarray(
            np.concatenate(
                [np.asarray(w_qkv[:, cq], f), np.asarray(w_qkv[:, ck], f)], axis=1
            )
        ),
        "w_v": np.ascontiguousarray(np.asarray(w_qkv[:, cv], f)),
        "w_p": np.ascontiguousarray(np.asarray(w_proj[g * 256:(g + 1) * 256, :], f)),
        "b_q": np.ascontiguousarray(np.asarray(b_qkv[cq], f)),
        "b_v2": np.ascontiguousarray(np.asarray(b_qkv[cv], f).reshape(4, 64).T),
    }


def kernel(x, w_qkv, b_qkv, w_proj, b_proj):
    from concourse.bass_utils import run_bass_kernel_spmd

    x = np.asarray(x, np.float32)
    B, n_tok, _ = x.shape
    nc = _get_built(n_tok)

    in_maps = [
        make_in_map(x[core // 4], w_qkv, b_qkv, w_proj, core % 4)
        for core in range(N_CORES)
    ]
    res = run_bass_kernel_spmd(
        nc, in_maps, core_ids=list(range(N_CORES)), trace=TRACE
    )
    global LAST_RESULTS
    LAST_RESULTS = res
    outs = [r["out"] for r in res.results]
    bp = np.asarray(b_proj, np.float32)
    full = np.stack(
        [
            outs[4 * b] + outs[4 * b + 1] + outs[4 * b + 2] + outs[4 * b + 3] + bp
            for b in range(B)
        ]
    )
    return full.astype(np.float32)


# revision 15
# speedup vs baseline: 2.9205x; 1.2277x over previous
"""Tensor-parallel multi-head attention for Trainium2 (8 NeuronCores).

Problem: nn_MultiHeadAttention (B=2, N=2048, C=1024, H=16, D=64), fp32.

Sharding: core = batch * 4 + head_group; each core handles 1 batch and 4
heads (tensor-parallel over heads, data-parallel over batch). Each core
computes its heads' QKV projections, attention, and a *partial* output
projection (its 256 rows of w_proj); the host sums the 4 partials per
batch and adds b_proj.

Kernel math notes:
  - x is transposed on the host to xT [C, N] (feature-major) so all
    projections contract over partitions.
  - Scores are computed transposed: sT[m, n] = k[m]·q[n] with keys m on
    partitions -- so P@V needs no on-chip transposes. Two heads run
    concurrently on the PE array via row-tiling (K=64 each).
  - Softmax: no max-subtraction (logits ~ N(0,1), exp is fp32-safe);
    denominator obtained by appending a ones-column to V (row 64 of the
    attention-output accumulator); probabilities are normalized after
    the P@V matmul via a reciprocal + DMA partition-broadcast multiply.
  - k-bias is mathematically softmax-invariant and dropped; v-bias is
    added to the attention output (softmax rows sum to 1); q-bias is
    applied at QKV eviction; proj-bias is added on the host.
"""

import numpy as np
from contextlib import ExitStack

P = 128
C = 1024
D = 64
N_CORES = 8

_BUILT = {}
TRACE = False   # set True (e.g. from test.py) to capture an NTFF profile
LAST_RESULTS = None  # BassKernelResults of the most recent kernel() call


def _build(n_tok, debug=False):
    import concourse.bass as bass
    import concourse.mybir as mybir
    import concourse.tile as tile
    from concourse import bacc
    from concourse.bass import ts

    fp32 = mybir.dt.float32
    fp32r = mybir.dt.float32r
    Exp = mybir.ActivationFunctionType.Exp
    mult = mybir.AluOpType.mult

    NC5 = n_tok // 512  # 512-wide query chunks
    MC = n_tok // 128   # 128-wide key chunks
    CC = C // P         # contraction chunks for projections

    nc = bacc.Bacc("TRN2", target_bir_lowering=False, debug=debug)

    xt_d = nc.dram_tensor("xt", [C, n_tok], fp32, kind="ExternalInput").ap()
    wqk_d = nc.dram_tensor("w_qk", [C, 512], fp32, kind="ExternalInput").ap()
    wv_d = nc.dram_tensor("w_v", [C, 256], fp32, kind="ExternalInput").ap()
    wp_d = nc.dram_tensor("w_p", [256, C], fp32, kind="ExternalInput").ap()
    bq_d = nc.dram_tensor("b_q", [256], fp32, kind="ExternalInput").ap()
    bv_d = nc.dram_tensor("b_v2", [64, 4], fp32, kind="ExternalInput").ap()
    out_d = nc.dram_tensor("out", [n_tok, C], fp32, kind="ExternalOutput").ap()

    with tile.TileContext(nc) as tc, ExitStack() as ctx:
        persist = ctx.enter_context(tc.tile_pool(name="persist", bufs=1))
        p_pool = ctx.enter_context(tc.tile_pool(name="p_pool", bufs=3))
        ev_pool = ctx.enter_context(tc.tile_pool(name="ev_pool", bufs=2))
        ph1_cm = tc.tile_pool(name="ph1", bufs=1)
        ph1 = ph1_cm.__enter__()
        s_pool = ctx.enter_context(tc.tile_pool(name="s", bufs=2, space="PSUM"))
        sm_pool = ctx.enter_context(tc.tile_pool(name="sm", bufs=2, space="PSUM"))
        o2_pool = ctx.enter_context(tc.tile_pool(name="o2", bufs=2, space="PSUM"))
        dram_pool = ctx.enter_context(tc.tile_pool(name="dram", bufs=4, space="DRAM"))

        xt = ph1.tile([P, CC, n_tok], fp32r)
        wqk = ph1.tile([P, CC, 512], fp32r)
        wv = ph1.tile([P, CC, 256], fp32r)
        bq = ph1.tile([P, 2], fp32)
        wp = persist.tile([P, 2, C], fp32r)
        bv = persist.tile([64, 4], fp32)
        qk = persist.tile([P, 4, n_tok], fp32r)   # jc: 0,1 = qT pairs, 2,3 = kT pairs
        vsb = persist.tile([P, MC, 4, 65], fp32r)  # token-major V + ones column
        o2n = persist.tile([P, 2, n_tok], fp32r)   # normalized attn out, feature-major

        # per-chunk loads so the first matmuls start after ~1/8 of the data
        xt_src = xt_d.rearrange("(co p) n -> p co n", p=P)
        wqk_src = wqk_d.rearrange("(co p) j -> p co j", p=P)
        wv_src = wv_d.rearrange("(co p) j -> p co j", p=P)
        for cc in range(CC):
            nc.gpsimd.dma_start(wqk[:, cc], wqk_src[:, cc])
            nc.gpsimd.dma_start(xt[:, cc], xt_src[:, cc])
        for cc in range(CC):
            nc.gpsimd.dma_start(wv[:, cc], wv_src[:, cc])
        nc.gpsimd.dma_start(wp[:], wp_d.rearrange("(pc p) e -> p pc e", p=P))
        nc.sync.dma_start(bq[:], bq_d.rearrange("(c p) -> p c", p=P))
        nc.sync.dma_start(bv[:], bv_d)
        ones = persist.tile([P, 1], fp32)
        nc.vector.memset(ones[:], 1.0)
        nc.vector.tensor_copy(
            out=vsb[:, :, :, 64:65],
            in_=ones[:, None, :, None].to_broadcast((P, MC, 4, 1)),
        )

        def emit_qk(pc):
            # kT then qT for this head pair (k first: scores need all keys)
            for jc, wcol in ((2 + pc, 256 + pc * 128), (pc, pc * 128)):
                for n5 in range(NC5):
                    ps = sm_pool.tile([P, 512], fp32, tag="sm")
                    for cc in range(CC):
                        nc.tensor.matmul(
                            ps[:],
                            wqk[:, cc, wcol:wcol + 128],
                            xt[:, cc, ts(n5, 512)],
                            start=(cc == 0),
                            stop=(cc == CC - 1),
                        )
                    if jc < 2:  # q: add bias
                        nc.vector.tensor_scalar_add(
                            qk[:, jc, ts(n5, 512)], ps[:], bq[:, jc:jc + 1]
                        )
                    else:  # k: bias dropped (softmax-invariant)
                        nc.vector.tensor_copy(
                            out=qk[:, jc, ts(n5, 512)], in_=ps[:]
                        )

        def emit_v():
            for nt in range(MC):
                psv = sm_pool.tile([P, 256], fp32, tag="sm")
                for cc in range(CC):
                    nc.tensor.matmul(
                        psv[:],
                        xt[:, cc, ts(nt, 128)],
                        wv[:, cc, :],
                        start=(cc == 0),
                        stop=(cc == CC - 1),
                    )
                nc.vector.tensor_copy(
                    out=vsb[:, nt, :, 0:64],
                    in_=psv[:].rearrange("p (h d) -> p h d", d=64),
                )

        def emit_attn(pc):
            for n5 in range(NC5):
                o2a = o2_pool.tile([65, 512], fp32, tag="o2")
                o2b = o2_pool.tile([65, 512], fp32, tag="o2")
                for mc in range(MC):
                    s = s_pool.tile([P, 1024], fp32, tag="s")
                    nc.tensor.matmul(
                        s[:, 0:512],
                        qk[0:64, 2 + pc, ts(mc, 128)],
                        qk[0:64, pc, ts(n5, 512)],
                        start=True, stop=True, tile_position=(0, 0),
                    )
                    nc.tensor.matmul(
                        s[:, 512:1024],
                        qk[64:128, 2 + pc, ts(mc, 128)],
                        qk[64:128, pc, ts(n5, 512)],
                        start=True, stop=True, tile_position=(64, 0),
                    )
                    pab = p_pool.tile([P, 1024], fp32r, tag="pab")
                    nc.scalar.activation(pab[:], s[:], Exp, scale=0.125)
                    nc.tensor.matmul(
                        o2a[:], vsb[:, mc, 2 * pc, 0:65], pab[:, 0:512],
                        start=(mc == 0), stop=(mc == MC - 1),
                    )
                    nc.tensor.matmul(
                        o2b[:], vsb[:, mc, 2 * pc + 1, 0:65], pab[:, 512:1024],
                        start=(mc == 0), stop=(mc == MC - 1),
                    )
                for hl, o2 in ((0, o2a), (1, o2b)):
                    # free the psum bank with one copy; normalize lazily
                    o2s = ev_pool.tile([65, 512], fp32, tag="o2s")
                    nc.vector.tensor_copy(out=o2s[:], in_=o2[:])
                    # partition-broadcast the denominator row via a DRAM
                    # bounce, then reciprocal across all 64 lanes
                    rd = dram_pool.tile([1, 512], fp32, tag="rd")
                    nc.sync.dma_start(rd[:], o2s[64:65, :])
                    rb = ev_pool.tile([64, 512], fp32, tag="rb")
                    rd_bcast = bass.AP(
                        tensor=rd.tensor, offset=rd.offset, ap=[[0, 64], [1, 512]]
                    )
                    nc.sync.dma_start(rb[:], rd_bcast)
                    nc.vector.reciprocal(rb[:], rb[:])
                    if hl == 0:
                        dst = o2n[0:64, pc, ts(n5, 512)]
                        nc.vector.scalar_tensor_tensor(
                            dst, o2s[0:64, :], 1.0, rb[:], op0=mult, op1=mult
                        )
                        nc.vector.tensor_scalar_add(
                            dst, dst, bv[:, 2 * pc:2 * pc + 1]
                        )
                    else:
                        stg = ev_pool.tile([64, 512], fp32r, tag="stg")
                        nc.vector.scalar_tensor_tensor(
                            stg[:], o2s[0:64, :], 1.0, rb[:], op0=mult, op1=mult
                        )
                        nc.vector.tensor_scalar_add(
                            stg[:], stg[:], bv[:, 2 * pc + 1:2 * pc + 2]
                        )
                        nc.sync.dma_start(o2n[64:128, pc, ts(n5, 512)], stg[:])

        def emit_proj(ostg):
            for nt in range(MC):
                po = ostg.tile([P, 1024], fp32, tag="po")
                for ec in range(2):
                    pp = sm_pool.tile([P, 512], fp32, tag="sm")
                    for pc in range(2):
                        nc.tensor.matmul(
                            pp[:],
                            o2n[:, pc, ts(nt, 128)],
                            wp[:, pc, ts(ec, 512)],
                            start=(pc == 0),
                            stop=(pc == 1),
                        )
                    nc.vector.tensor_copy(out=po[:, ts(ec, 512)], in_=pp[:])
                nc.sync.dma_start(out_d[ts(nt, 128), :], po[:])

        emit_qk(0)
        emit_v()
        emit_attn(0)
        emit_qk(1)
        ph1_cm.__exit__(None, None, None)  # free xt/weights space for ostg
        emit_attn(1)
        with tc.tile_pool(name="ostg", bufs=2) as ostg:
            emit_proj(ostg)

    nc.compile()
    return nc


def _get_built(n_tok):
    if n_tok not in _BUILT:
        _BUILT[n_tok] = _build(n_tok)
    return _BUILT[n_tok]


def make_in_map(x_b, w_qkv, b_qkv, w_proj, g):
    """Per-core input shards: batch slice x_b, head-group g (4 heads)."""
    f = np.float32
    cq = slice(g * 256, g * 256 + 256)
    ck = slice(C + g * 256, C + g * 256 + 256)
    cv = slice(2 * C + g * 256, 2 * C + g * 256 + 256)
    return {
        "xt": np.ascontiguousarray(np.asarray(x_b, f).T),
        "w_qk": np.ascontiguousarray(
            np.concatenate(
                [np.asarray(w_qkv[:, cq], f), np.asarray(w_qkv[:, ck], f)], axis=1
            )
        ),
        "w_v": np.ascontiguousarray(np.asarray(w_qkv[:, cv], f)),
        "w_p": np.ascontiguousarray(np.asarray(w_proj[g * 256:(g + 1) * 256, :], f)),
        "b_q": np.ascontiguousarray(np.asarray(b_qkv[cq], f)),
        "b_v2": np.ascontiguousarray(np.asarray(b_qkv[cv], f).reshape(4, 64).T),
    }


def kernel(x, w_qkv, b_qkv, w_proj, b_proj):
    from concourse.bass_utils import run_bass_kernel_spmd

    x = np.asarray(x, np.float32)
    B, n_tok, _ = x.shape
    nc = _get_built(n_tok)

    in_maps = [
        make_in_map(x[core // 4], w_qkv, b_qkv, w_proj, core % 4)
        for core in range(N_CORES)
    ]
    res = run_bass_kernel_spmd(
        nc, in_maps, core_ids=list(range(N_CORES)), trace=TRACE
    )
    global LAST_RESULTS
    LAST_RESULTS = res
    outs = [r["out"] for r in res.results]
    bp = np.asarray(b_proj, np.float32)
    full = np.stack(
        [
            outs[4 * b] + outs[4 * b + 1] + outs[4 * b + 2] + outs[4 * b + 3] + bp
            for b in range(B)
        ]
    )
    return full.astype(np.float32)
